# revision 15
# baseline (speedup 1.0000x reference)
"""Causal self-attention (B=4, T=2048, C=2048, H=16, RoPE) on 8 trn2 cores.

Sharding: core c -> (batch b = c//2, head-group g = c%2), 8 heads per core.
Each core computes y_partial[b] = attn_heads(g) @ W_proj[rows(g)]; the host
sums the two partials per batch.

Fully-fused single-TileContext design (v1):
 - All matmul operands are bf16 (host-cast); PSUM accumulation is fp32.
 - x (8 MB), v (4 MB) and O^T (4 MB) stay resident in SBUF: q/k/v never
   round-trip through DRAM, there is no phase barrier, and the PE queue
   stays deep across projection -> attention -> output-projection.
 - RoPE: pair-swap permutation matmul + bf16 DVE combine (4x DVE mode).
 - Scores as S^T tiles [128 k, 512 q]; exp on ScalarE (no max subtraction
   needed; |scores*scale| < ~10); causal mask as bf16 multiply on DVE.
 - Softmax denominators: bf16 DVE accumulation of exp tiles + one
   ones-vector matmul per q-block; 1/sum broadcast across partitions via a
   [1x128] ones stationary matmul (no DRAM round trip).
 - Output projection reads resident O^T; y leaves as bf16 and the host
   upcasts + sums the two head-group partials per batch.
"""
import sys

sys.path.insert(0, "/opt/trn_rl_repo")

import numpy as np

B, T, C, H, D = 4, 2048, 2048, 16, 128
G = 2                      # head groups (tensor-parallel dim)
HG = H // G                # heads per core = 8
CG = HG * D                # channels per group = 1024
P = 128
NQ = T // 512              # q chunks of 512
KO = C // P                # contraction chunks = 16
ROPE_BASE = 10000.0
SCALE = 1.0 / float(np.sqrt(D))
N_CORES = 8

_cached = None


def _build_program(reps=1, phases="all", variant="full", bench_mode=False):
    import concourse.bass as bass
    import concourse.tile as tile
    from concourse import bacc, mybir

    f32 = mybir.dt.float32
    f32r = mybir.dt.float32r
    bf16 = mybir.dt.bfloat16
    Exp = mybir.ActivationFunctionType.Exp

    pesums = (variant == "pesums")
    # normalization modes: "pebcast" broadcasts 1/rowsum across partitions
    # with a [1x128] ones matmul; "dmabcast" uses a DRAM round-trip broadcast
    # DMA; "nonorm" skips normalization (debug only).
    norm = "dmabcast"
    if variant in ("nonorm", "pebcast"):
        norm = variant

    nc = bacc.Bacc()

    # host-prepacked inputs (bf16 unless noted)
    xq_d = nc.declare_dram_parameter("xq", [P, KO, T], bf16, isOutput=False)
    wq_d = nc.declare_dram_parameter("wq", [HG, P, KO, D], bf16, isOutput=False)
    wk_d = nc.declare_dram_parameter("wk", [HG, P, KO, D], bf16, isOutput=False)
    wv_d = nc.declare_dram_parameter("wv", [4, P, KO, 256], bf16, isOutput=False)
    wp_d = nc.declare_dram_parameter("wp", [8, P, HG, 256], bf16, isOutput=False)
    cos_d = nc.declare_dram_parameter("cosT", [P, T], bf16, isOutput=False)
    sin_d = nc.declare_dram_parameter("sinT", [P, T], bf16, isOutput=False)
    swp_d = nc.declare_dram_parameter("swapT", [P, P], bf16, isOutput=False)
    onc_d = nc.declare_dram_parameter("onesc", [P, 1], bf16, isOutput=False)
    onr_d = nc.declare_dram_parameter("onesr", [1, P], f32, isOutput=False)
    mask_d = nc.declare_dram_parameter("masks", [P, 4, 512], bf16, isOutput=False)
    if bench_mode:
        # identical device work, but y goes to scratch and only a tiny token
        # is an ExternalOutput -> host transfer floor vanishes for timing
        y_d = nc.dram_tensor("y_scratch", [T, C], bf16)
        tok_d = nc.declare_dram_parameter("tok", [P, P], bf16, isOutput=True)
    else:
        y_d = nc.declare_dram_parameter("y", [T, C], bf16, isOutput=True)
        tok_d = None
    inv_s = (nc.dram_tensor("inv_s", [HG, NQ, 1, 512], f32)
             if norm == "dmabcast" else None)

    for _rep in range(reps):
        with tile.TileContext(nc) as tc:
            with tc.tile_pool(name="const", bufs=1) as cp, \
                 tc.tile_pool(name="xpool", bufs=1) as xp, \
                 tc.tile_pool(name="vpool", bufs=1) as vp, \
                 tc.tile_pool(name="opool", bufs=1) as oap, \
                 tc.tile_pool(name="wvp", bufs=1) as wvp, \
                 tc.tile_pool(name="wqk", bufs=2) as wqkp, \
                 tc.tile_pool(name="qkt", bufs=1) as qkp, \
                 tc.tile_pool(name="rope", bufs=2) as rp, \
                 tc.tile_pool(name="ptp", bufs=4) as ptp, \
                 tc.tile_pool(name="smp", bufs=2) as smp, \
                 tc.tile_pool(name="wpp", bufs=2) as wpp, \
                 tc.tile_pool(name="yp", bufs=2) as yp, \
                 tc.tile_pool(name="psA", bufs=2, space="PSUM") as psA, \
                 tc.tile_pool(name="psS", bufs=3, space="PSUM") as psS, \
                 tc.tile_pool(name="psO", bufs=2, space="PSUM") as psO, \
                 tc.tile_pool(name="psN", bufs=1, space="PSUM") as psN:

                cosT = cp.tile([P, T], bf16)
                sinT = cp.tile([P, T], bf16)
                swpT = cp.tile([P, P], bf16)
                masks = cp.tile([P, 4, 512], bf16)
                onesc = cp.tile([P, 1], bf16)
                onesr = cp.tile([1, P], f32r)

                x_sb = xp.tile([P, KO, T], bf16)
                v_sb = vp.tile([P, KO, CG], bf16)
                o_all = oap.tile([P, HG, T], bf16)

                # critical-path-first DMA order: x streams in T-quarters so
                # the first chains (which only read early t columns) unblock
                # after ~1/4 of x has landed; then constants and weights.
                nc.sync.dma_start(swpT[:], swp_d.ap())
                for tq in range(4):
                    tsl = slice(tq * 512, (tq + 1) * 512)
                    for ki in range(KO):
                        nc.sync.dma_start(x_sb[:, ki, tsl], xq_d.ap()[:, ki, tsl])
                nc.sync.dma_start(cosT[:], cos_d.ap())
                nc.sync.dma_start(sinT[:], sin_d.ap())
                nc.sync.dma_start(masks[:], mask_d.ap())
                nc.sync.dma_start(onesc[:], onc_d.ap())
                nc.sync.dma_start(onesr[:], onr_d.ap().bitcast(f32r))

                def v_chains(cc):
                    # v columns [cc*256, (cc+1)*256) = heads {2cc, 2cc+1}
                    wvc = wvp.tile([P, KO, 256], bf16, tag="wv")
                    nc.sync.dma_start(wvc[:, :KO // 2, :], wv_d.ap()[cc, :, :KO // 2, :])
                    nc.sync.dma_start(wvc[:, KO // 2:, :], wv_d.ap()[cc, :, KO // 2:, :])
                    for tb in range(16):
                        ps = psA.tile([P, 512], f32, tag="a")
                        for ki in range(KO):
                            nc.tensor.matmul(ps[:, :256],
                                             x_sb[:, ki, tb * P:(tb + 1) * P],
                                             wvc[:, ki, :],
                                             start=(ki == 0), stop=(ki == KO - 1))
                        nc.scalar.copy(v_sb[:, tb, cc * 256:(cc + 1) * 256], ps[:, :256])

                def qk_proj(h, tc, wt, dst):
                    ps = psA.tile([P, 512], f32, tag="a")
                    for ki in range(KO):
                        nc.tensor.matmul(ps[:], wt[:, ki, :],
                                         x_sb[:, ki, tc * 512:(tc + 1) * 512],
                                         start=(ki == 0), stop=(ki == KO - 1))
                    raw = rp.tile([P, 512], bf16, tag="raw")
                    nc.scalar.copy(raw[:], ps[:])
                    ps2 = psS.tile([P, 512], f32, tag="s")
                    nc.tensor.matmul(ps2[:], swpT[:], raw[:], start=True, stop=True)
                    sw = rp.tile([P, 512], bf16, tag="sw")
                    nc.scalar.copy(sw[:], ps2[:])
                    tA = rp.tile([P, 512], bf16, tag="tA")
                    nc.vector.tensor_mul(tA[:], raw[:], cosT[:, tc * 512:(tc + 1) * 512])
                    tB = rp.tile([P, 512], bf16, tag="tB")
                    nc.vector.tensor_mul(tB[:], sw[:], sinT[:, tc * 512:(tc + 1) * 512])
                    nc.vector.tensor_add(dst[:, tc * 512:(tc + 1) * 512], tA[:], tB[:])

                def attention(h, qb, qt, kt):
                    nkb = 4 * (qb + 1)
                    LAG = 2
                    ps_o = psO.tile([P, 512], f32, tag="o")
                    ptsum = smp.tile([P, 512], bf16, tag="ptsum")
                    if pesums:
                        ps_n = psN.tile([1, 512], f32, tag="n")
                    pend = []

                    def flush_one():
                        kb0, pt0, off = pend.pop(0)
                        W = 512 - off
                        nc.tensor.matmul(ps_o[:, off:], v_sb[:, kb0, h * D:(h + 1) * D],
                                         pt0[:, :W], start=(kb0 == 0),
                                         stop=(kb0 == nkb - 1))
                        if pesums:
                            nc.tensor.matmul(ps_n[:, off:], onesc[:], pt0[:, :W],
                                             start=(kb0 == 0), stop=(kb0 == nkb - 1))
                        elif kb0 == 0:
                            nc.vector.tensor_copy(ptsum[:], pt0[:])
                        else:
                            nc.vector.tensor_add(ptsum[:, off:], ptsum[:, off:],
                                                 pt0[:, :W])

                    for kb in range(nkb):
                        j = kb - 4 * qb
                        # diagonal block j: query columns below j*128 are fully
                        # masked -- skip them (scores/exp/mask/attnV all narrow)
                        off = j * P if j > 0 else 0
                        W = 512 - off
                        ps_s = psS.tile([P, 512], f32, tag="s")
                        nc.tensor.matmul(ps_s[:, :W], kt[:, kb * P:(kb + 1) * P],
                                         qt[:, qb * 512 + off:(qb + 1) * 512],
                                         start=True, stop=True)
                        pt = ptp.tile([P, 512], bf16, tag="pt")
                        nc.scalar.activation(pt[:, :W], ps_s[:, :W], Exp, scale=SCALE)
                        if j >= 0:  # diagonal block: causal mask
                            nc.vector.tensor_mul(pt[:, :P], pt[:, :P],
                                                 masks[:, j, off:off + P])
                        pend.append((kb, pt, off))
                        if len(pend) > LAG:
                            flush_one()
                    while pend:
                        flush_one()

                    if norm == "nonorm":
                        nc.scalar.copy(o_all[:, h, qb * 512:(qb + 1) * 512], ps_o[:])
                        return
                    if not pesums:
                        ps_n = psN.tile([1, 512], f32, tag="n")
                        nc.tensor.matmul(ps_n[:], onesc[:], ptsum[:],
                                         start=True, stop=True)
                    if norm == "pebcast":
                        inv = smp.tile([1, 512], f32r, tag="inv")
                        with nc.allow_low_precision(reason="f32r is bitwise fp32"):
                            nc.vector.reciprocal(inv[:], ps_n[:])
                        ps_b = psS.tile([P, 512], f32, tag="s")
                        nc.tensor.matmul(ps_b[:], onesr[:], inv[:],
                                         start=True, stop=True)
                        invb = smp.tile([P, 512], bf16, tag="invb")
                        nc.scalar.copy(invb[:], ps_b[:])
                    else:  # dmabcast
                        inv = smp.tile([1, 512], f32, tag="inv")
                        nc.vector.reciprocal(inv[:], ps_n[:])
                        nc.gpsimd.dma_start(inv_s.ap()[h, qb], inv[:])
                        invb = smp.tile([P, 512], f32, tag="invb")
                        nc.gpsimd.dma_start(
                            invb[:], inv_s.ap()[h, qb].to_broadcast((P, 512)))
                    nc.vector.tensor_mul(o_all[:, h, qb * 512:(qb + 1) * 512],
                                         ps_o[:], invb[:])

                # ---- emission: v by column pairs, heads pipelined ----
                v_chains(0)
                qt = kt = None
                for h in range(HG):
                    if h in (2, 4, 6):
                        v_chains(h // 2)
                    wtq = wqkp.tile([P, KO, D], bf16, tag="wq")
                    nc.sync.dma_start(wtq[:], wq_d.ap()[h])
                    wtk = wqkp.tile([P, KO, D], bf16, tag="wk")
                    nc.sync.dma_start(wtk[:], wk_d.ap()[h])
                    qt = qkp.tile([P, T], bf16, tag="qt")
                    kt = qkp.tile([P, T], bf16, tag="kt")
                    for tc in range(NQ):
                        qk_proj(h, tc, wtq, qt)
                        qk_proj(h, tc, wtk, kt)
                        if phases == "all" and tc >= 1:
                            attention(h, tc - 1, qt, kt)
                    if phases == "all":
                        attention(h, NQ - 1, qt, kt)

                # ---- output projection from resident O^T ----
                if phases == "all":
                    for co in range(C // 256):
                        wpc = wpp.tile([P, HG, 256], bf16, tag="wp")
                        nc.sync.dma_start(wpc[:], wp_d.ap()[co])
                        for qc in range(T // P):
                            ps = psA.tile([P, 512], f32, tag="a")
                            for hh in range(HG):
                                nc.tensor.matmul(ps[:, :256],
                                                 o_all[:, hh, qc * P:(qc + 1) * P],
                                                 wpc[:, hh, :],
                                                 start=(hh == 0), stop=(hh == HG - 1))
                            ysb = yp.tile([P, 256], bf16, tag="ysb")
                            nc.scalar.copy(ysb[:], ps[:, :256])
                            nc.sync.dma_start(
                                y_d.ap()[qc * P:(qc + 1) * P, co * 256:(co + 1) * 256],
                                ysb[:])
                            if bench_mode and co == C // 256 - 1 and qc == T // P - 1:
                                nc.sync.dma_start(tok_d.ap(), ysb[:, :P])
                else:
                    if bench_mode:
                        nc.sync.dma_start(tok_d.ap(), kt[:, :P])

    nc.finalize()
    return nc


def _host_tables():
    import ml_dtypes
    bf16 = ml_dtypes.bfloat16
    thetas = 1.0 / (ROPE_BASE ** (np.arange(0, D, 2, dtype=np.float32) / D))  # [64]
    t = np.arange(T, dtype=np.float32)
    freqs = t[None, :] * thetas[:, None]                     # [64, T]
    cosT = np.repeat(np.cos(freqs), 2, axis=0).astype(bf16)  # [128, T]
    sinT = np.repeat(np.sin(freqs), 2, axis=0).astype(bf16)
    swapT = np.zeros((P, P), np.float32)
    for i in range(0, P, 2):
        swapT[i, i + 1] = 1.0      # (S^T)[2i, 2i+1] = +1
        swapT[i + 1, i] = -1.0     # (S^T)[2i+1, 2i] = -1
    onesc = np.ones((P, 1), bf16)
    onesr = np.ones((1, P), np.float32)
    ki = np.arange(P)[:, None]
    qi = np.arange(512)[None, :]
    masks = np.stack([(ki + 128 * j <= qi).astype(bf16) for j in range(4)],
                     axis=1)  # [128, 4, 512]
    return (cosT, sinT, swapT.astype(bf16), onesc, onesr,
            np.ascontiguousarray(masks))


class _Runner:
    """Compile the bass program to a PJRT executable once; rerun cheaply.

    Mirrors concourse.bass2jax.run_bass_via_pjrt but caches the jitted
    shard_map callable so repeated kernel() calls (and benchmarking) do not
    pay tracing + compile again.
    """

    def __init__(self, nc):
        import jax
        from jax.sharding import Mesh, PartitionSpec
        try:
            from jax.experimental.shard_map import shard_map
        except ImportError:
            from jax import shard_map
        from concourse import bass2jax, mybir

        bass2jax.install_neuronx_cc_hook()
        self.jax = jax
        self.nc = nc
        assert nc.dbg_addr is None or not nc.dbg_callbacks
        partition_name = (nc.partition_id_tensor.name
                          if nc.partition_id_tensor else None)

        in_names, out_names, out_avals, zero_shapes = [], [], [], []
        for alloc in nc.m.functions[0].allocations:
            if not isinstance(alloc, mybir.MemoryLocationSet):
                continue
            name = alloc.memorylocations[0].name
            if alloc.kind == "ExternalInput":
                if name != partition_name and name != (
                        nc.dbg_addr.name if nc.dbg_addr else None):
                    in_names.append(name)
            elif alloc.kind == "ExternalOutput":
                shape = tuple(alloc.tensor_shape)
                dtype = mybir.dt.np(alloc.dtype)
                out_names.append(name)
                out_avals.append(jax.core.ShapedArray(shape, dtype))
                zero_shapes.append((shape, dtype))
        self.in_names, self.out_names = in_names, out_names
        self.out_avals, self.zero_shapes = out_avals, zero_shapes
        n_params, n_outs = len(in_names), len(out_names)
        self.n_params = n_params

        all_names = list(in_names) + list(out_names)
        if nc.dbg_addr is not None:
            all_names.append(nc.dbg_addr.name)
        if partition_name is not None:
            all_names.append(partition_name)

        def _body(*args):
            operands = list(args)
            if nc.dbg_addr is not None:
                operands.append(jax.numpy.zeros((1, 2), "uint32"))
            if partition_name is not None:
                operands.append(bass2jax.partition_id_tensor())
            outs = bass2jax._bass_exec_p.bind(
                *operands,
                out_avals=tuple(out_avals),
                in_names=tuple(all_names),
                out_names=tuple(out_names),
                lowering_input_output_aliases=(),
                sim_require_finite=True,
                sim_require_nnan=True,
                nc=nc,
            )
            return tuple(outs)

        devices = jax.devices()[:N_CORES]
        self.mesh = Mesh(np.asarray(devices), ("core",))
        self.pspec = PartitionSpec("core")
        in_specs = (self.pspec,) * (n_params + n_outs)
        out_specs = (self.pspec,) * n_outs
        donate = tuple(range(n_params, n_params + n_outs))
        self.fn = jax.jit(
            shard_map(_body, mesh=self.mesh, in_specs=in_specs,
                      out_specs=out_specs, check_rep=False),
            donate_argnums=donate, keep_unused=True)

    def concat_inputs(self, in_maps):
        return [np.concatenate([np.asarray(in_maps[c][n])
                                for c in range(N_CORES)], axis=0)
                for n in self.in_names]

    def device_inputs(self, concat_in):
        from jax.sharding import NamedSharding
        sh = NamedSharding(self.mesh, self.pspec)
        return [self.jax.device_put(a, sh) for a in concat_in]

    def zeros(self, on_device=False):
        zs = [np.zeros((N_CORES * s[0], *s[1:]), d) for s, d in self.zero_shapes]
        if on_device:
            from jax.sharding import NamedSharding
            sh = NamedSharding(self.mesh, self.pspec)
            zs = [self.jax.device_put(z, sh) for z in zs]
        return zs

    def run(self, args):
        out_arrs = self.fn(*args)
        return [
            {n: np.asarray(out_arrs[i]).reshape(N_CORES, *self.out_avals[i].shape)[c]
             for i, n in enumerate(self.out_names)}
            for c in range(N_CORES)
        ]


_runner = None


def _get_runner():
    global _cached, _runner
    if _runner is None:
        if _cached is None:
            _cached = _build_program(variant="dvesums")
        _runner = _Runner(_cached)
    return _runner


def _make_in_maps(x, W_qkv, W_proj):
    import ml_dtypes
    bf16 = ml_dtypes.bfloat16
    cosT, sinT, swapT, onesc, onesr, masks = _host_tables()
    in_maps = []
    for c in range(N_CORES):
        b, g = c // G, c % G
        cols = slice(g * CG, (g + 1) * CG)
        xT = x[b].T  # [C, T]
        wq = W_qkv[:, 0 * C:1 * C][:, cols]
        wk = W_qkv[:, 1 * C:2 * C][:, cols]
        wv = W_qkv[:, 2 * C:3 * C][:, cols]
        wpm = W_proj[g * CG:(g + 1) * CG, :]
        in_maps.append({
            # [C, T] -> [p, ko, T]
            "xq": np.ascontiguousarray(
                xT.reshape(KO, P, T).transpose(1, 0, 2).astype(bf16)),
            # [C, CG] -> [h, p, ko, D]
            "wq": np.ascontiguousarray(
                wq.reshape(KO, P, HG, D).transpose(2, 1, 0, 3).astype(bf16)),
            "wk": np.ascontiguousarray(
                wk.reshape(KO, P, HG, D).transpose(2, 1, 0, 3).astype(bf16)),
            # [C, CG] -> [cc, p, ko, 256]
            "wv": np.ascontiguousarray(
                wv.reshape(KO, P, 4, 256).transpose(2, 1, 0, 3).astype(bf16)),
            # [CG, C] -> [co, p, hb, 256]
            "wp": np.ascontiguousarray(
                wpm.reshape(HG, P, 8, 256).transpose(2, 1, 0, 3).astype(bf16)),
            "cosT": cosT, "sinT": sinT, "swapT": swapT,
            "onesc": onesc, "onesr": onesr, "masks": masks,
        })
    return in_maps


def kernel(x, W_qkv, W_proj):
    x = np.asarray(x, dtype=np.float32)
    W_qkv = np.asarray(W_qkv, dtype=np.float32)
    W_proj = np.asarray(W_proj, dtype=np.float32)

    r = _get_runner()
    concat_in = r.concat_inputs(_make_in_maps(x, W_qkv, W_proj))
    results = r.run(concat_in + r.zeros())
    out = np.empty((B, T, C), np.float32)
    for b in range(B):
        out[b] = (results[2 * b]["y"].astype(np.float32)
                  + results[2 * b + 1]["y"].astype(np.float32))
    return out


# revision 16
# speedup vs baseline: 1.1434x; 1.1434x over previous
"""Causal self-attention (B=4, T=2048, C=2048, H=16, RoPE) on 8 trn2 cores.

Sharding: core c -> (batch b = c//2, head-group g = c%2), 8 heads per core.
Each core computes y_partial[b] = attn_heads(g) @ W_proj[rows(g)]; the host
sums the two partials per batch.

Fully-fused single-TileContext design (v1):
 - All matmul operands are bf16 (host-cast); PSUM accumulation is fp32.
 - x (8 MB), v (4 MB) and O^T (4 MB) stay resident in SBUF: q/k/v never
   round-trip through DRAM, there is no phase barrier, and the PE queue
   stays deep across projection -> attention -> output-projection.
 - RoPE: pair-swap permutation matmul + bf16 DVE combine (4x DVE mode).
 - Scores as S^T tiles [128 k, 512 q]; exp on ScalarE (no max subtraction
   needed; |scores*scale| < ~10); causal mask as bf16 multiply on DVE.
 - Softmax denominators: bf16 DVE accumulation of exp tiles + one
   ones-vector matmul per q-block; 1/sum broadcast across partitions via a
   [1x128] ones stationary matmul (no DRAM round trip).
 - Output projection reads resident O^T; y leaves as bf16 and the host
   upcasts + sums the two head-group partials per batch.
"""
import sys

sys.path.insert(0, "/opt/trn_rl_repo")

import numpy as np

B, T, C, H, D = 4, 2048, 2048, 16, 128
G = 2                      # head groups (tensor-parallel dim)
HG = H // G                # heads per core = 8
CG = HG * D                # channels per group = 1024
P = 128
NQ = T // 512              # q chunks of 512
KO = C // P                # contraction chunks = 16
ROPE_BASE = 10000.0
SCALE = 1.0 / float(np.sqrt(D))
N_CORES = 8

_cached = None


def _build_program(reps=1, phases="all", variant="full", bench_mode=False):
    import concourse.bass as bass
    import concourse.tile as tile
    from concourse import bacc, mybir

    f32 = mybir.dt.float32
    f32r = mybir.dt.float32r
    bf16 = mybir.dt.bfloat16
    Exp = mybir.ActivationFunctionType.Exp

    pesums = (variant == "pesums")
    # normalization modes: "pebcast" broadcasts 1/rowsum across partitions
    # with a [1x128] ones matmul; "dmabcast" uses a DRAM round-trip broadcast
    # DMA; "nonorm" skips normalization (debug only).
    norm = "dmabcast"
    if variant in ("nonorm", "pebcast"):
        norm = variant

    nc = bacc.Bacc()

    # host-prepacked inputs (bf16 unless noted)
    xq_d = nc.declare_dram_parameter("xq", [P, KO, T], bf16, isOutput=False)
    wq_d = nc.declare_dram_parameter("wq", [HG, P, KO, D], bf16, isOutput=False)
    wk_d = nc.declare_dram_parameter("wk", [HG, P, KO, D], bf16, isOutput=False)
    wv_d = nc.declare_dram_parameter("wv", [4, P, KO, 256], bf16, isOutput=False)
    wp_d = nc.declare_dram_parameter("wp", [8, P, HG, 256], bf16, isOutput=False)
    cos_d = nc.declare_dram_parameter("cosT", [P, T], bf16, isOutput=False)
    sin_d = nc.declare_dram_parameter("sinT", [P, T], bf16, isOutput=False)
    swp_d = nc.declare_dram_parameter("swapT", [P, P], bf16, isOutput=False)
    onc_d = nc.declare_dram_parameter("onesc", [P, 1], bf16, isOutput=False)
    onr_d = nc.declare_dram_parameter("onesr", [1, P], f32, isOutput=False)
    mask_d = nc.declare_dram_parameter("masks", [P, 4, 512], bf16, isOutput=False)
    if bench_mode:
        # identical device work, but y goes to scratch and only a tiny token
        # is an ExternalOutput -> host transfer floor vanishes for timing
        y_d = nc.dram_tensor("y_scratch", [T, C], bf16)
        tok_d = nc.declare_dram_parameter("tok", [P, P], bf16, isOutput=True)
    else:
        y_d = nc.declare_dram_parameter("y", [T, C], bf16, isOutput=True)
        tok_d = None
    inv_s = (nc.dram_tensor("inv_s", [HG, NQ, 1, 512], f32)
             if norm == "dmabcast" else None)

    for _rep in range(reps):
        with tile.TileContext(nc) as tc:
            with tc.tile_pool(name="const", bufs=1) as cp, \
                 tc.tile_pool(name="xpool", bufs=1) as xp, \
                 tc.tile_pool(name="vpool", bufs=1) as vp, \
                 tc.tile_pool(name="opool", bufs=1) as oap, \
                 tc.tile_pool(name="wvp", bufs=1) as wvp, \
                 tc.tile_pool(name="wqk", bufs=2) as wqkp, \
                 tc.tile_pool(name="qkt", bufs=1) as qkp, \
                 tc.tile_pool(name="rope", bufs=2) as rp, \
                 tc.tile_pool(name="ptp", bufs=4) as ptp, \
                 tc.tile_pool(name="smp", bufs=2) as smp, \
                 tc.tile_pool(name="wpp", bufs=2) as wpp, \
                 tc.tile_pool(name="yp", bufs=2) as yp, \
                 tc.tile_pool(name="psA", bufs=2, space="PSUM") as psA, \
                 tc.tile_pool(name="psS", bufs=3, space="PSUM") as psS, \
                 tc.tile_pool(name="psO", bufs=2, space="PSUM") as psO, \
                 tc.tile_pool(name="psN", bufs=1, space="PSUM") as psN:

                cosT = cp.tile([P, T], bf16)
                sinT = cp.tile([P, T], bf16)
                swpT = cp.tile([P, P], bf16)
                masks = cp.tile([P, 4, 512], bf16)
                onesc = cp.tile([P, 1], bf16)
                onesr = cp.tile([1, P], f32r)

                x_sb = xp.tile([P, KO, T], bf16)
                v_sb = vp.tile([P, KO, CG], bf16)
                o_all = oap.tile([P, HG, T], bf16)

                # critical-path-first DMA order: x chunks, first weights,
                # constants, then the rest stream in under compute.
                nc.sync.dma_start(x_sb[:, 0, :], xq_d.ap()[:, 0, :])
                nc.sync.dma_start(swpT[:], swp_d.ap())
                for ki in range(1, KO):
                    nc.sync.dma_start(x_sb[:, ki, :], xq_d.ap()[:, ki, :])
                nc.sync.dma_start(cosT[:], cos_d.ap())
                nc.sync.dma_start(sinT[:], sin_d.ap())
                nc.sync.dma_start(masks[:], mask_d.ap())
                nc.sync.dma_start(onesc[:], onc_d.ap())
                nc.sync.dma_start(onesr[:], onr_d.ap().bitcast(f32r))

                def v_chains(cc):
                    # v columns [cc*256, (cc+1)*256) = heads {2cc, 2cc+1}
                    wvc = wvp.tile([P, KO, 256], bf16, tag="wv")
                    nc.sync.dma_start(wvc[:, :KO // 2, :], wv_d.ap()[cc, :, :KO // 2, :])
                    nc.sync.dma_start(wvc[:, KO // 2:, :], wv_d.ap()[cc, :, KO // 2:, :])
                    for tb in range(16):
                        ps = psA.tile([P, 512], f32, tag="a")
                        for ki in range(KO):
                            nc.tensor.matmul(ps[:, :256],
                                             x_sb[:, ki, tb * P:(tb + 1) * P],
                                             wvc[:, ki, :],
                                             start=(ki == 0), stop=(ki == KO - 1))
                        nc.scalar.copy(v_sb[:, tb, cc * 256:(cc + 1) * 256], ps[:, :256])

                def qk_proj(h, tc, wt, dst):
                    ps = psA.tile([P, 512], f32, tag="a")
                    for ki in range(KO):
                        nc.tensor.matmul(ps[:], wt[:, ki, :],
                                         x_sb[:, ki, tc * 512:(tc + 1) * 512],
                                         start=(ki == 0), stop=(ki == KO - 1))
                    raw = rp.tile([P, 512], bf16, tag="raw")
                    nc.scalar.copy(raw[:], ps[:])
                    ps2 = psS.tile([P, 512], f32, tag="s")
                    nc.tensor.matmul(ps2[:], swpT[:], raw[:], start=True, stop=True)
                    sw = rp.tile([P, 512], bf16, tag="sw")
                    nc.scalar.copy(sw[:], ps2[:])
                    tA = rp.tile([P, 512], bf16, tag="tA")
                    nc.vector.tensor_mul(tA[:], raw[:], cosT[:, tc * 512:(tc + 1) * 512])
                    tB = rp.tile([P, 512], bf16, tag="tB")
                    nc.vector.tensor_mul(tB[:], sw[:], sinT[:, tc * 512:(tc + 1) * 512])
                    nc.vector.tensor_add(dst[:, tc * 512:(tc + 1) * 512], tA[:], tB[:])

                def attention(h, qb, qt, kt):
                    nkb = 4 * (qb + 1)
                    LAG = 2
                    ps_o = psO.tile([P, 512], f32, tag="o")
                    ptsum = smp.tile([P, 512], bf16, tag="ptsum")
                    if pesums:
                        ps_n = psN.tile([1, 512], f32, tag="n")
                    pend = []

                    def flush_one():
                        kb0, pt0, off = pend.pop(0)
                        W = 512 - off
                        nc.tensor.matmul(ps_o[:, off:], v_sb[:, kb0, h * D:(h + 1) * D],
                                         pt0[:, :W], start=(kb0 == 0),
                                         stop=(kb0 == nkb - 1))
                        if pesums:
                            nc.tensor.matmul(ps_n[:, off:], onesc[:], pt0[:, :W],
                                             start=(kb0 == 0), stop=(kb0 == nkb - 1))
                        elif kb0 == 0:
                            nc.vector.tensor_copy(ptsum[:], pt0[:])
                        else:
                            nc.vector.tensor_add(ptsum[:, off:], ptsum[:, off:],
                                                 pt0[:, :W])

                    for kb in range(nkb):
                        j = kb - 4 * qb
                        # diagonal block j: query columns below j*128 are fully
                        # masked -- skip them (scores/exp/mask/attnV all narrow)
                        off = j * P if j > 0 else 0
                        W = 512 - off
                        ps_s = psS.tile([P, 512], f32, tag="s")
                        nc.tensor.matmul(ps_s[:, :W], kt[:, kb * P:(kb + 1) * P],
                                         qt[:, qb * 512 + off:(qb + 1) * 512],
                                         start=True, stop=True)
                        pt = ptp.tile([P, 512], bf16, tag="pt")
                        nc.scalar.activation(pt[:, :W], ps_s[:, :W], Exp, scale=SCALE)
                        if j >= 0:  # diagonal block: causal mask
                            nc.vector.tensor_mul(pt[:, :P], pt[:, :P],
                                                 masks[:, j, off:off + P])
                        pend.append((kb, pt, off))
                        if len(pend) > LAG:
                            flush_one()
                    while pend:
                        flush_one()

                    if norm == "nonorm":
                        nc.scalar.copy(o_all[:, h, qb * 512:(qb + 1) * 512], ps_o[:])
                        return
                    if not pesums:
                        ps_n = psN.tile([1, 512], f32, tag="n")
                        nc.tensor.matmul(ps_n[:], onesc[:], ptsum[:],
                                         start=True, stop=True)
                    if norm == "pebcast":
                        inv = smp.tile([1, 512], f32r, tag="inv")
                        with nc.allow_low_precision(reason="f32r is bitwise fp32"):
                            nc.vector.reciprocal(inv[:], ps_n[:])
                        ps_b = psS.tile([P, 512], f32, tag="s")
                        nc.tensor.matmul(ps_b[:], onesr[:], inv[:],
                                         start=True, stop=True)
                        invb = smp.tile([P, 512], bf16, tag="invb")
                        nc.scalar.copy(invb[:], ps_b[:])
                    else:  # dmabcast
                        inv = smp.tile([1, 512], f32, tag="inv")
                        nc.vector.reciprocal(inv[:], ps_n[:])
                        nc.gpsimd.dma_start(inv_s.ap()[h, qb], inv[:])
                        invb = smp.tile([P, 512], f32, tag="invb")
                        nc.gpsimd.dma_start(
                            invb[:], inv_s.ap()[h, qb].to_broadcast((P, 512)))
                    nc.vector.tensor_mul(o_all[:, h, qb * 512:(qb + 1) * 512],
                                         ps_o[:], invb[:])

                # ---- emission: v by column pairs, heads pipelined ----
                v_chains(0)
                qt = kt = None
                for h in range(HG):
                    if h in (2, 4, 6):
                        v_chains(h // 2)
                    wtq = wqkp.tile([P, KO, D], bf16, tag="wq")
                    nc.sync.dma_start(wtq[:], wq_d.ap()[h])
                    wtk = wqkp.tile([P, KO, D], bf16, tag="wk")
                    nc.sync.dma_start(wtk[:], wk_d.ap()[h])
                    qt = qkp.tile([P, T], bf16, tag="qt")
                    kt = qkp.tile([P, T], bf16, tag="kt")
                    for tc in range(NQ):
                        qk_proj(h, tc, wtq, qt)
                        qk_proj(h, tc, wtk, kt)
                        if phases == "all" and tc >= 1:
                            attention(h, tc - 1, qt, kt)
                    if phases == "all":
                        attention(h, NQ - 1, qt, kt)

                # ---- output projection from resident O^T ----
                if phases == "all":
                    for co in range(C // 256):
                        wpc = wpp.tile([P, HG, 256], bf16, tag="wp")
                        nc.sync.dma_start(wpc[:], wp_d.ap()[co])
                        for qc in range(T // P):
                            ps = psA.tile([P, 512], f32, tag="a")
                            for hh in range(HG):
                                nc.tensor.matmul(ps[:, :256],
                                                 o_all[:, hh, qc * P:(qc + 1) * P],
                                                 wpc[:, hh, :],
                                                 start=(hh == 0), stop=(hh == HG - 1))
                            ysb = yp.tile([P, 256], bf16, tag="ysb")
                            nc.scalar.copy(ysb[:], ps[:, :256])
                            nc.sync.dma_start(
                                y_d.ap()[qc * P:(qc + 1) * P, co * 256:(co + 1) * 256],
                                ysb[:])
                            if bench_mode and co == C // 256 - 1 and qc == T // P - 1:
                                nc.sync.dma_start(tok_d.ap(), ysb[:, :P])
                else:
                    if bench_mode:
                        nc.sync.dma_start(tok_d.ap(), kt[:, :P])

    nc.finalize()
    return nc


def _host_tables():
    import ml_dtypes
    bf16 = ml_dtypes.bfloat16
    thetas = 1.0 / (ROPE_BASE ** (np.arange(0, D, 2, dtype=np.float32) / D))  # [64]
    t = np.arange(T, dtype=np.float32)
    freqs = t[None, :] * thetas[:, None]                     # [64, T]
    cosT = np.repeat(np.cos(freqs), 2, axis=0).astype(bf16)  # [128, T]
    sinT = np.repeat(np.sin(freqs), 2, axis=0).astype(bf16)
    swapT = np.zeros((P, P), np.float32)
    for i in range(0, P, 2):
        swapT[i, i + 1] = 1.0      # (S^T)[2i, 2i+1] = +1
        swapT[i + 1, i] = -1.0     # (S^T)[2i+1, 2i] = -1
    onesc = np.ones((P, 1), bf16)
    onesr = np.ones((1, P), np.float32)
    ki = np.arange(P)[:, None]
    qi = np.arange(512)[None, :]
    masks = np.stack([(ki + 128 * j <= qi).astype(bf16) for j in range(4)],
                     axis=1)  # [128, 4, 512]
    return (cosT, sinT, swapT.astype(bf16), onesc, onesr,
            np.ascontiguousarray(masks))


class _Runner:
    """Compile the bass program to a PJRT executable once; rerun cheaply.

    Mirrors concourse.bass2jax.run_bass_via_pjrt but caches the jitted
    shard_map callable so repeated kernel() calls (and benchmarking) do not
    pay tracing + compile again.
    """

    def __init__(self, nc):
        import jax
        from jax.sharding import Mesh, PartitionSpec
        try:
            from jax.experimental.shard_map import shard_map
        except ImportError:
            from jax import shard_map
        from concourse import bass2jax, mybir

        bass2jax.install_neuronx_cc_hook()
        self.jax = jax
        self.nc = nc
        assert nc.dbg_addr is None or not nc.dbg_callbacks
        partition_name = (nc.partition_id_tensor.name
                          if nc.partition_id_tensor else None)

        in_names, out_names, out_avals, zero_shapes = [], [], [], []
        for alloc in nc.m.functions[0].allocations:
            if not isinstance(alloc, mybir.MemoryLocationSet):
                continue
            name = alloc.memorylocations[0].name
            if alloc.kind == "ExternalInput":
                if name != partition_name and name != (
                        nc.dbg_addr.name if nc.dbg_addr else None):
                    in_names.append(name)
            elif alloc.kind == "ExternalOutput":
                shape = tuple(alloc.tensor_shape)
                dtype = mybir.dt.np(alloc.dtype)
                out_names.append(name)
                out_avals.append(jax.core.ShapedArray(shape, dtype))
                zero_shapes.append((shape, dtype))
        self.in_names, self.out_names = in_names, out_names
        self.out_avals, self.zero_shapes = out_avals, zero_shapes
        n_params, n_outs = len(in_names), len(out_names)
        self.n_params = n_params

        all_names = list(in_names) + list(out_names)
        if nc.dbg_addr is not None:
            all_names.append(nc.dbg_addr.name)
        if partition_name is not None:
            all_names.append(partition_name)

        def _body(*args):
            operands = list(args)
            if nc.dbg_addr is not None:
                operands.append(jax.numpy.zeros((1, 2), "uint32"))
            if partition_name is not None:
                operands.append(bass2jax.partition_id_tensor())
            outs = bass2jax._bass_exec_p.bind(
                *operands,
                out_avals=tuple(out_avals),
                in_names=tuple(all_names),
                out_names=tuple(out_names),
                lowering_input_output_aliases=(),
                sim_require_finite=True,
                sim_require_nnan=True,
                nc=nc,
            )
            return tuple(outs)

        devices = jax.devices()[:N_CORES]
        self.mesh = Mesh(np.asarray(devices), ("core",))
        self.pspec = PartitionSpec("core")
        in_specs = (self.pspec,) * (n_params + n_outs)
        out_specs = (self.pspec,) * n_outs
        donate = tuple(range(n_params, n_params + n_outs))
        self.fn = jax.jit(
            shard_map(_body, mesh=self.mesh, in_specs=in_specs,
                      out_specs=out_specs, check_rep=False),
            donate_argnums=donate, keep_unused=True)

    def concat_inputs(self, in_maps):
        return [np.concatenate([np.asarray(in_maps[c][n])
                                for c in range(N_CORES)], axis=0)
                for n in self.in_names]

    def device_inputs(self, concat_in):
        from jax.sharding import NamedSharding
        sh = NamedSharding(self.mesh, self.pspec)
        return [self.jax.device_put(a, sh) for a in concat_in]

    def zeros(self, on_device=False):
        zs = [np.zeros((N_CORES * s[0], *s[1:]), d) for s, d in self.zero_shapes]
        if on_device:
            from jax.sharding import NamedSharding
            sh = NamedSharding(self.mesh, self.pspec)
            zs = [self.jax.device_put(z, sh) for z in zs]
        return zs

    def run(self, args):
        out_arrs = self.fn(*args)
        return [
            {n: np.asarray(out_arrs[i]).reshape(N_CORES, *self.out_avals[i].shape)[c]
             for i, n in enumerate(self.out_names)}
            for c in range(N_CORES)
        ]


_runner = None


def _get_runner():
    global _cached, _runner
    if _runner is None:
        if _cached is None:
            _cached = _build_program(variant="dvesums")
        _runner = _Runner(_cached)
    return _runner


def _make_in_maps(x, W_qkv, W_proj):
    import ml_dtypes
    bf16 = ml_dtypes.bfloat16
    cosT, sinT, swapT, onesc, onesr, masks = _host_tables()
    in_maps = []
    for c in range(N_CORES):
        b, g = c // G, c % G
        cols = slice(g * CG, (g + 1) * CG)
        xT = x[b].T  # [C, T]
        wq = W_qkv[:, 0 * C:1 * C][:, cols]
        wk = W_qkv[:, 1 * C:2 * C][:, cols]
        wv = W_qkv[:, 2 * C:3 * C][:, cols]
        wpm = W_proj[g * CG:(g + 1) * CG, :]
        in_maps.append({
            # [C, T] -> [p, ko, T]
            "xq": np.ascontiguousarray(
                xT.reshape(KO, P, T).transpose(1, 0, 2).astype(bf16)),
            # [C, CG] -> [h, p, ko, D]
            "wq": np.ascontiguousarray(
                wq.reshape(KO, P, HG, D).transpose(2, 1, 0, 3).astype(bf16)),
            "wk": np.ascontiguousarray(
                wk.reshape(KO, P, HG, D).transpose(2, 1, 0, 3).astype(bf16)),
            # [C, CG] -> [cc, p, ko, 256]
            "wv": np.ascontiguousarray(
                wv.reshape(KO, P, 4, 256).transpose(2, 1, 0, 3).astype(bf16)),
            # [CG, C] -> [co, p, hb, 256]
            "wp": np.ascontiguousarray(
                wpm.reshape(HG, P, 8, 256).transpose(2, 1, 0, 3).astype(bf16)),
            "cosT": cosT, "sinT": sinT, "swapT": swapT,
            "onesc": onesc, "onesr": onesr, "masks": masks,
        })
    return in_maps


def kernel(x, W_qkv, W_proj):
    x = np.asarray(x, dtype=np.float32)
    W_qkv = np.asarray(W_qkv, dtype=np.float32)
    W_proj = np.asarray(W_proj, dtype=np.float32)

    r = _get_runner()
    concat_in = r.concat_inputs(_make_in_maps(x, W_qkv, W_proj))
    results = r.run(concat_in + r.zeros())
    out = np.empty((B, T, C), np.float32)
    for b in range(B):
        out[b] = (results[2 * b]["y"].astype(np.float32)
                  + results[2 * b + 1]["y"].astype(np.float32))
    return out


# revision 25
# speedup vs baseline: 1.3601x; 1.1896x over previous
"""Causal self-attention (B=4, T=2048, C=2048, H=16, RoPE) on 8 trn2 cores.

Sharding: core c -> (batch b = c//2, head-group g = c%2), 8 heads per core.
Each core computes y_partial[b] = attn_heads(g) @ W_proj[rows(g)]; the host
sums the two partials per batch.

Fully-fused single-TileContext design (v1):
 - All matmul operands are bf16 (host-cast); PSUM accumulation is fp32.
 - x (8 MB), v (4 MB) and O^T (4 MB) stay resident in SBUF: q/k/v never
   round-trip through DRAM, there is no phase barrier, and the PE queue
   stays deep across projection -> attention -> output-projection.
 - RoPE: pair-swap permutation matmul + bf16 DVE combine (4x DVE mode).
 - Scores as S^T tiles [128 k, 512 q]; exp on ScalarE (no max subtraction
   needed; |scores*scale| < ~10); causal mask as bf16 multiply on DVE.
 - Softmax denominators: bf16 DVE accumulation of exp tiles + one
   ones-vector matmul per q-block; 1/sum broadcast across partitions via a
   [1x128] ones stationary matmul (no DRAM round trip).
 - Output projection reads resident O^T; y leaves as bf16 and the host
   upcasts + sums the two head-group partials per batch.
"""
import sys

sys.path.insert(0, "/opt/trn_rl_repo")

import numpy as np

B, T, C, H, D = 4, 2048, 2048, 16, 128
G = 2                      # head groups (tensor-parallel dim)
HG = H // G                # heads per core = 8
CG = HG * D                # channels per group = 1024
P = 128
NQ = T // 512              # q chunks of 512
KO = C // P                # contraction chunks = 16
ROPE_BASE = 10000.0
SCALE = 1.0 / float(np.sqrt(D))
N_CORES = 8

_cached = None


def _build_program(reps=1, phases="all", variant="full", bench_mode=False):
    import concourse.bass as bass
    import concourse.tile as tile
    from concourse import bacc, mybir

    f32 = mybir.dt.float32
    f32r = mybir.dt.float32r
    bf16 = mybir.dt.bfloat16
    Exp = mybir.ActivationFunctionType.Exp

    pesums = (variant == "pesums")
    # normalization modes: "pebcast" broadcasts 1/rowsum across partitions
    # with a [1x128] ones matmul; "dmabcast" uses a DRAM round-trip broadcast
    # DMA; "nonorm" skips normalization (debug only).
    norm = "dmabcast"
    if variant in ("nonorm", "pebcast"):
        norm = variant

    nc = bacc.Bacc()

    # host-prepacked inputs (bf16 unless noted)
    xq_d = nc.declare_dram_parameter("xq", [P, KO, T], bf16, isOutput=False)
    wq_d = nc.declare_dram_parameter("wq", [HG, P, KO, D], bf16, isOutput=False)
    wk_d = nc.declare_dram_parameter("wk", [HG, P, KO, D], bf16, isOutput=False)
    wv_d = nc.declare_dram_parameter("wv", [4, P, KO, 256], bf16, isOutput=False)
    wp_d = nc.declare_dram_parameter("wp", [8, P, HG, 256], bf16, isOutput=False)
    cos_d = nc.declare_dram_parameter("cosT", [P, T], bf16, isOutput=False)
    sin_d = nc.declare_dram_parameter("sinT", [P, T], bf16, isOutput=False)
    swp_d = nc.declare_dram_parameter("swapT", [P, P], bf16, isOutput=False)
    onc_d = nc.declare_dram_parameter("onesc", [P, 1], bf16, isOutput=False)
    onr_d = nc.declare_dram_parameter("onesr", [1, P], f32, isOutput=False)
    mask_d = nc.declare_dram_parameter("masks", [P, P], bf16, isOutput=False)
    if bench_mode:
        # identical device work, but y goes to scratch and only a tiny token
        # is an ExternalOutput -> host transfer floor vanishes for timing
        y_d = nc.dram_tensor("y_scratch", [T, C], bf16)
        tok_d = nc.declare_dram_parameter("tok", [P, P], bf16, isOutput=True)
    else:
        y_d = nc.declare_dram_parameter("y", [T, C], bf16, isOutput=True)
        tok_d = None
    inv_s = (nc.dram_tensor("inv_s", [HG, NQ, 1, 512], f32)
             if norm == "dmabcast" else None)

    for _rep in range(reps):
        with tile.TileContext(nc) as tc:
            with tc.tile_pool(name="const", bufs=1) as cp, \
                 tc.tile_pool(name="xpool", bufs=1) as xp, \
                 tc.tile_pool(name="vpool", bufs=1) as vp, \
                 tc.tile_pool(name="opool", bufs=1) as oap, \
                 tc.tile_pool(name="wvp", bufs=1) as wvp, \
                 tc.tile_pool(name="wqk", bufs=2) as wqkp, \
                 tc.tile_pool(name="qkt", bufs=2) as qkp, \
                 tc.tile_pool(name="rope", bufs=2) as rp, \
                 tc.tile_pool(name="ptp", bufs=4) as ptp, \
                 tc.tile_pool(name="smp", bufs=2) as smp, \
                 tc.tile_pool(name="wpp", bufs=2) as wpp, \
                 tc.tile_pool(name="yp", bufs=2) as yp, \
                 tc.tile_pool(name="psA", bufs=2, space="PSUM") as psA, \
                 tc.tile_pool(name="psS", bufs=3, space="PSUM") as psS, \
                 tc.tile_pool(name="psO", bufs=2, space="PSUM") as psO, \
                 tc.tile_pool(name="psN", bufs=1, space="PSUM") as psN:

                cosT = cp.tile([P, T], bf16)
                sinT = cp.tile([P, T], bf16)
                swpT = cp.tile([P, P], bf16)
                masks = cp.tile([P, P], bf16)
                onesc = cp.tile([P, 1], bf16)
                onesr = cp.tile([1, P], f32r)

                x_sb = xp.tile([P, KO, T], bf16)
                v_sb = vp.tile([P, KO, CG], bf16)
                o_all = oap.tile([P, HG, T], bf16)

                # critical-path-first DMA order: x chunks, first weights,
                # constants, then the rest stream in under compute.
                nc.sync.dma_start(x_sb[:, 0, :], xq_d.ap()[:, 0, :])
                nc.sync.dma_start(swpT[:], swp_d.ap())
                for ki in range(1, KO):
                    nc.sync.dma_start(x_sb[:, ki, :], xq_d.ap()[:, ki, :])
                nc.sync.dma_start(cosT[:], cos_d.ap())
                nc.sync.dma_start(sinT[:], sin_d.ap())
                nc.sync.dma_start(masks[:], mask_d.ap())
                nc.sync.dma_start(onesc[:], onc_d.ap())
                nc.sync.dma_start(onesr[:], onr_d.ap().bitcast(f32r))

                def v_chains(cc):
                    # v columns [cc*256, (cc+1)*256) = heads {2cc, 2cc+1}
                    wvc = wvp.tile([P, KO, 256], bf16, tag="wv")
                    nc.sync.dma_start(wvc[:, :KO // 2, :], wv_d.ap()[cc, :, :KO // 2, :])
                    nc.sync.dma_start(wvc[:, KO // 2:, :], wv_d.ap()[cc, :, KO // 2:, :])
                    for tb in range(16):
                        ps = psA.tile([P, 512], f32, tag="a")
                        for ki in range(KO):
                            nc.tensor.matmul(ps[:, :256],
                                             x_sb[:, ki, tb * P:(tb + 1) * P],
                                             wvc[:, ki, :],
                                             start=(ki == 0), stop=(ki == KO - 1))
                        nc.scalar.copy(v_sb[:, tb, cc * 256:(cc + 1) * 256], ps[:, :256])

                def qk_proj(h, tc, wt, dst):
                    ps = psA.tile([P, 512], f32, tag="a")
                    for ki in range(KO):
                        nc.tensor.matmul(ps[:], wt[:, ki, :],
                                         x_sb[:, ki, tc * 512:(tc + 1) * 512],
                                         start=(ki == 0), stop=(ki == KO - 1))
                    raw = rp.tile([P, 512], bf16, tag="raw")
                    nc.scalar.copy(raw[:], ps[:])
                    ps2 = psS.tile([P, 512], f32, tag="s")
                    nc.tensor.matmul(ps2[:], swpT[:], raw[:], start=True, stop=True)
                    sw = rp.tile([P, 512], bf16, tag="sw")
                    nc.scalar.copy(sw[:], ps2[:])
                    tA = rp.tile([P, 512], bf16, tag="tA", bufs=1)
                    nc.vector.tensor_mul(tA[:], raw[:], cosT[:, tc * 512:(tc + 1) * 512])
                    tB = rp.tile([P, 512], bf16, tag="tB", bufs=1)
                    nc.vector.tensor_mul(tB[:], sw[:], sinT[:, tc * 512:(tc + 1) * 512])
                    nc.vector.tensor_add(dst[:, tc * 512:(tc + 1) * 512], tA[:], tB[:])

                def attention(h, qb, qt, kt):
                    nkb = 4 * (qb + 1)
                    LAG = 2
                    ps_o = psO.tile([P, 512], f32, tag="o")
                    ptsum = smp.tile([P, 512], bf16, tag="ptsum")
                    if pesums:
                        ps_n = psN.tile([1, 512], f32, tag="n")
                    pend = []

                    def flush_one():
                        kb0, pt0, off = pend.pop(0)
                        W = 512 - off
                        nc.tensor.matmul(ps_o[:, off:], v_sb[:, kb0, h * D:(h + 1) * D],
                                         pt0[:, :W], start=(kb0 == 0),
                                         stop=(kb0 == nkb - 1))
                        if pesums:
                            nc.tensor.matmul(ps_n[:, off:], onesc[:], pt0[:, :W],
                                             start=(kb0 == 0), stop=(kb0 == nkb - 1))
                        elif kb0 == 0:
                            nc.vector.tensor_copy(ptsum[:], pt0[:])
                        else:
                            nc.vector.tensor_add(ptsum[:, off:], ptsum[:, off:],
                                                 pt0[:, :W])

                    for kb in range(nkb):
                        j = kb - 4 * qb
                        # diagonal block j: query columns below j*128 are fully
                        # masked -- skip them (scores/exp/mask/attnV all narrow)
                        off = j * P if j > 0 else 0
                        W = 512 - off
                        ps_s = psS.tile([P, 512], f32, tag="s")
                        nc.tensor.matmul(ps_s[:, :W], kt[:, kb * P:(kb + 1) * P],
                                         qt[:, qb * 512 + off:(qb + 1) * 512],
                                         start=True, stop=True)
                        pt = ptp.tile([P, 512], bf16, tag="pt")
                        nc.scalar.activation(pt[:, :W], ps_s[:, :W], Exp, scale=SCALE)
                        if j >= 0:  # diagonal block: triangular causal mask
                            nc.vector.tensor_mul(pt[:, :P], pt[:, :P], masks[:])
                        pend.append((kb, pt, off))
                        if len(pend) > LAG:
                            flush_one()
                    while pend:
                        flush_one()

                    if norm == "nonorm":
                        nc.scalar.copy(o_all[:, h, qb * 512:(qb + 1) * 512], ps_o[:])
                        return
                    if not pesums:
                        ps_n = psN.tile([1, 512], f32, tag="n")
                        nc.tensor.matmul(ps_n[:], onesc[:], ptsum[:],
                                         start=True, stop=True)
                    if norm == "pebcast":
                        inv = smp.tile([1, 512], f32r, tag="inv")
                        with nc.allow_low_precision(reason="f32r is bitwise fp32"):
                            nc.vector.reciprocal(inv[:], ps_n[:])
                        ps_b = psS.tile([P, 512], f32, tag="s")
                        nc.tensor.matmul(ps_b[:], onesr[:], inv[:],
                                         start=True, stop=True)
                        invb = smp.tile([P, 512], bf16, tag="invb")
                        nc.scalar.copy(invb[:], ps_b[:])
                    else:  # dmabcast
                        inv = smp.tile([1, 512], f32, tag="inv", bufs=1)
                        nc.vector.reciprocal(inv[:], ps_n[:])
                        nc.gpsimd.dma_start(inv_s.ap()[h, qb], inv[:])
                        invb = smp.tile([P, 512], f32, tag="invb")
                        nc.gpsimd.dma_start(
                            invb[:], inv_s.ap()[h, qb].to_broadcast((P, 512)))
                    nc.vector.tensor_mul(o_all[:, h, qb * 512:(qb + 1) * 512],
                                         ps_o[:], invb[:])

                # ---- emission: v by column pairs, heads pipelined.  Each
                # head's last attention block is carried across the head
                # boundary so the next head's projection matmuls give the PE
                # filler work while that block's exps catch up on ScalarE.
                v_chains(0)
                qt = kt = None
                pending_att = None
                for h in range(HG):
                    if h in (2, 4, 6):
                        v_chains(h // 2)
                    wtq = wqkp.tile([P, KO, D], bf16, tag="wq")
                    nc.sync.dma_start(wtq[:], wq_d.ap()[h])
                    wtk = wqkp.tile([P, KO, D], bf16, tag="wk")
                    nc.sync.dma_start(wtk[:], wk_d.ap()[h])
                    qt = qkp.tile([P, T], bf16, tag="qt")
                    kt = qkp.tile([P, T], bf16, tag="kt")
                    for tc in range(NQ):
                        qk_proj(h, tc, wtq, qt)
                        qk_proj(h, tc, wtk, kt)
                        if phases == "all":
                            if tc == 0 and pending_att is not None:
                                attention(*pending_att)
                                pending_att = None
                            if tc >= 1:
                                attention(h, tc - 1, qt, kt)
                    if phases == "all":
                        pending_att = (h, NQ - 1, qt, kt)
                if phases == "all" and pending_att is not None:
                    attention(*pending_att)
                    pending_att = None

                # ---- output projection from resident O^T ----
                if phases == "all":
                    for co in range(C // 256):
                        wpc = wpp.tile([P, HG, 256], bf16, tag="wp")
                        nc.sync.dma_start(wpc[:, :HG // 2, :], wp_d.ap()[co, :, :HG // 2, :])
                        nc.sync.dma_start(wpc[:, HG // 2:, :], wp_d.ap()[co, :, HG // 2:, :])
                        for qc in range(T // P):
                            ps = psA.tile([P, 512], f32, tag="a")
                            for hh in range(HG):
                                nc.tensor.matmul(ps[:, :256],
                                                 o_all[:, hh, qc * P:(qc + 1) * P],
                                                 wpc[:, hh, :],
                                                 start=(hh == 0), stop=(hh == HG - 1))
                            ysb = yp.tile([P, 256], bf16, tag="ysb")
                            nc.scalar.copy(ysb[:], ps[:, :256])
                            nc.sync.dma_start(
                                y_d.ap()[qc * P:(qc + 1) * P, co * 256:(co + 1) * 256],
                                ysb[:])
                            if bench_mode and co == C // 256 - 1 and qc == T // P - 1:
                                nc.sync.dma_start(tok_d.ap(), ysb[:, :P])
                else:
                    if bench_mode:
                        nc.sync.dma_start(tok_d.ap(), kt[:, :P])

    nc.finalize()
    return nc


def _host_tables():
    import ml_dtypes
    bf16 = ml_dtypes.bfloat16
    thetas = 1.0 / (ROPE_BASE ** (np.arange(0, D, 2, dtype=np.float32) / D))  # [64]
    t = np.arange(T, dtype=np.float32)
    freqs = t[None, :] * thetas[:, None]                     # [64, T]
    cosT = np.repeat(np.cos(freqs), 2, axis=0).astype(bf16)  # [128, T]
    sinT = np.repeat(np.sin(freqs), 2, axis=0).astype(bf16)
    swapT = np.zeros((P, P), np.float32)
    for i in range(0, P, 2):
        swapT[i, i + 1] = 1.0      # (S^T)[2i, 2i+1] = +1
        swapT[i + 1, i] = -1.0     # (S^T)[2i+1, 2i] = -1
    onesc = np.ones((P, 1), bf16)
    onesr = np.ones((1, P), np.float32)
    # diagonal blocks are computed from column-offset j*128 on, so a single
    # [128,128] lower-triangle (k_part <= q_local') serves every block
    ki = np.arange(P)[:, None]
    qi = np.arange(P)[None, :]
    masks = (ki <= qi).astype(bf16)  # [128, 128]
    return (cosT, sinT, swapT.astype(bf16), onesc, onesr,
            np.ascontiguousarray(masks))


class _Runner:
    """Compile the bass program to a PJRT executable once; rerun cheaply.

    Mirrors concourse.bass2jax.run_bass_via_pjrt but caches the jitted
    shard_map callable so repeated kernel() calls (and benchmarking) do not
    pay tracing + compile again.
    """

    def __init__(self, nc):
        import jax
        from jax.sharding import Mesh, PartitionSpec
        try:
            from jax.experimental.shard_map import shard_map
        except ImportError:
            from jax import shard_map
        from concourse import bass2jax, mybir

        bass2jax.install_neuronx_cc_hook()
        self.jax = jax
        self.nc = nc
        assert nc.dbg_addr is None or not nc.dbg_callbacks
        partition_name = (nc.partition_id_tensor.name
                          if nc.partition_id_tensor else None)

        in_names, out_names, out_avals, zero_shapes = [], [], [], []
        for alloc in nc.m.functions[0].allocations:
            if not isinstance(alloc, mybir.MemoryLocationSet):
                continue
            name = alloc.memorylocations[0].name
            if alloc.kind == "ExternalInput":
                if name != partition_name and name != (
                        nc.dbg_addr.name if nc.dbg_addr else None):
                    in_names.append(name)
            elif alloc.kind == "ExternalOutput":
                shape = tuple(alloc.tensor_shape)
                dtype = mybir.dt.np(alloc.dtype)
                out_names.append(name)
                out_avals.append(jax.core.ShapedArray(shape, dtype))
                zero_shapes.append((shape, dtype))
        self.in_names, self.out_names = in_names, out_names
        self.out_avals, self.zero_shapes = out_avals, zero_shapes
        n_params, n_outs = len(in_names), len(out_names)
        self.n_params = n_params

        all_names = list(in_names) + list(out_names)
        if nc.dbg_addr is not None:
            all_names.append(nc.dbg_addr.name)
        if partition_name is not None:
            all_names.append(partition_name)

        def _body(*args):
            operands = list(args)
            if nc.dbg_addr is not None:
                operands.append(jax.numpy.zeros((1, 2), "uint32"))
            if partition_name is not None:
                operands.append(bass2jax.partition_id_tensor())
            outs = bass2jax._bass_exec_p.bind(
                *operands,
                out_avals=tuple(out_avals),
                in_names=tuple(all_names),
                out_names=tuple(out_names),
                lowering_input_output_aliases=(),
                sim_require_finite=True,
                sim_require_nnan=True,
                nc=nc,
            )
            return tuple(outs)

        devices = jax.devices()[:N_CORES]
        self.mesh = Mesh(np.asarray(devices), ("core",))
        self.pspec = PartitionSpec("core")
        in_specs = (self.pspec,) * (n_params + n_outs)
        out_specs = (self.pspec,) * n_outs
        donate = tuple(range(n_params, n_params + n_outs))
        self.fn = jax.jit(
            shard_map(_body, mesh=self.mesh, in_specs=in_specs,
                      out_specs=out_specs, check_rep=False),
            donate_argnums=donate, keep_unused=True)

    def concat_inputs(self, in_maps):
        return [np.concatenate([np.asarray(in_maps[c][n])
                                for c in range(N_CORES)], axis=0)
                for n in self.in_names]

    def device_inputs(self, concat_in):
        from jax.sharding import NamedSharding
        sh = NamedSharding(self.mesh, self.pspec)
        return [self.jax.device_put(a, sh) for a in concat_in]

    def zeros(self, on_device=False):
        zs = [np.zeros((N_CORES * s[0], *s[1:]), d) for s, d in self.zero_shapes]
        if on_device:
            from jax.sharding import NamedSharding
            sh = NamedSharding(self.mesh, self.pspec)
            zs = [self.jax.device_put(z, sh) for z in zs]
        return zs

    def run(self, args):
        out_arrs = self.fn(*args)
        return [
            {n: np.asarray(out_arrs[i]).reshape(N_CORES, *self.out_avals[i].shape)[c]
             for i, n in enumerate(self.out_names)}
            for c in range(N_CORES)
        ]


_runner = None


def _get_runner():
    global _cached, _runner
    if _runner is None:
        if _cached is None:
            _cached = _build_program(variant="dvesums")
        _runner = _Runner(_cached)
    return _runner


def _make_in_maps(x, W_qkv, W_proj):
    import ml_dtypes
    bf16 = ml_dtypes.bfloat16
    cosT, sinT, swapT, onesc, onesr, masks = _host_tables()
    in_maps = []
    for c in range(N_CORES):
        b, g = c // G, c % G
        cols = slice(g * CG, (g + 1) * CG)
        xT = x[b].T  # [C, T]
        wq = W_qkv[:, 0 * C:1 * C][:, cols]
        wk = W_qkv[:, 1 * C:2 * C][:, cols]
        wv = W_qkv[:, 2 * C:3 * C][:, cols]
        wpm = W_proj[g * CG:(g + 1) * CG, :]
        in_maps.append({
            # [C, T] -> [p, ko, T]
            "xq": np.ascontiguousarray(
                xT.reshape(KO, P, T).transpose(1, 0, 2).astype(bf16)),
            # [C, CG] -> [h, p, ko, D]
            "wq": np.ascontiguousarray(
                wq.reshape(KO, P, HG, D).transpose(2, 1, 0, 3).astype(bf16)),
            "wk": np.ascontiguousarray(
                wk.reshape(KO, P, HG, D).transpose(2, 1, 0, 3).astype(bf16)),
            # [C, CG] -> [cc, p, ko, 256]
            "wv": np.ascontiguousarray(
                wv.reshape(KO, P, 4, 256).transpose(2, 1, 0, 3).astype(bf16)),
            # [CG, C] -> [co, p, hb, 256]
            "wp": np.ascontiguousarray(
                wpm.reshape(HG, P, 8, 256).transpose(2, 1, 0, 3).astype(bf16)),
            "cosT": cosT, "sinT": sinT, "swapT": swapT,
            "onesc": onesc, "onesr": onesr, "masks": masks,
        })
    return in_maps


def kernel(x, W_qkv, W_proj):
    x = np.asarray(x, dtype=np.float32)
    W_qkv = np.asarray(W_qkv, dtype=np.float32)
    W_proj = np.asarray(W_proj, dtype=np.float32)

    r = _get_runner()
    concat_in = r.concat_inputs(_make_in_maps(x, W_qkv, W_proj))
    results = r.run(concat_in + r.zeros())
    out = np.empty((B, T, C), np.float32)
    for b in range(B):
        out[b] = (results[2 * b]["y"].astype(np.float32)
                  + results[2 * b + 1]["y"].astype(np.float32))
    return out
